# revision 37
# baseline (speedup 1.0000x reference)
"""Causal single-head attention (B=8, T=2048, D=1024, HS=64) on 8 TRN2 NeuronCores.

Sharding: data-parallel over batch -- core b computes batch b end-to-end.
No collectives; outputs are concatenated on the host.

Compute path is bf16 (operands) with fp32 PSUM accumulation; softmax
denominator/normalization stays fp32. The host casts x/W to bf16 and
pre-swizzles both into their exact SBUF layouts so every DMA is a large
contiguous transfer; x^T rides the Sync HWDGE ring, weights/consts ride the
Scalar HWDGE ring in parallel.

Per-core pipeline, four 512-col t-supers, software-pipelined across supers:
  ~10 dep-free warmup matmuls keep the PE busy while the first DMAs land so
  the HAM activity monitor lifts the PE clock to 2.4 GHz before projection
  projection, W-stationary:  QKV^T[:,t] = W^T x^T
    half 0: [Wq/8 | Wk] (1/sqrt(HS) folded into Wq/bq)
    half 1: Wv col-packed -- chunk pairs run concurrently in the two column
    halves of the PE array (M=64 each, one shared PSUM bank; only the very
    first matmul clears has_written, so the odd group overwrites-where-clear
    onto a zeroed base)
  DVE evacuations: Q^T duplicated to partitions 64:128, K^T packed even/odd
  k-tile into partitions 0:64/64:128 (so the row-tiled S pair shares the
  array), V^T summed+biased -> PE-transpose -> V' [k, 65] (ones row from a
  GpSimd memset)
  attention in k-tile PAIRS: both S matmuls of a pair occupy disjoint row
  groups (contraction is 64) and run concurrently when the PE queue has
  backlog; causally-invalid columns are trimmed from S / exp / PV; one exp
  per pair (diagonal pairs write 128 causally-dead columns in the second S
  matmul so a single strided activation covers both tiles); the diagonal
  triangle is zeroed by GpSimd affine_select (off the DVE/ScalarE queues):
    S^T[k,q] = K-pair @ Q^T-super       (PSUM fp32, two banks, trimmed)
    P^T = exp(S^T)                      (ScalarE; logits ~N(0,1), no max sub)
    outT[65,q] += V'[k,65]^T @ P^T      (PSUM fp32 accumulate; row 64=denom)
  projection of super ts+1 is interleaved AHEAD of each PV in the pair loop
  of super ts, so the PE FIFO head never blocks on the exp chain; each
  super's epilogue is deferred into the next super's pipeline-fill window;
  wavefront: super 2's first four pairs ride inside attention(1) and super
  3's first half rides inside attention(2) (each needs only earlier supers'
  K/V), soaking up ScalarE idle between exp bursts -- super 3 accumulates
  into the retired projection PSUM bank; dep-free warm matmuls are woven
  between the DMA-paced projection chunks of supers 0-1 so the HAM never
  re-throttles; the last super's epilogue is split so subtiles 0-1 flush
  during the final exp.
  epilogue (fp32): PE-transpose outT back, DVE reciprocal + scale, DMA out.
"""

import sys

if "/opt/trn_rl_repo" not in sys.path:
    sys.path.insert(0, "/opt/trn_rl_repo")

import os
from contextlib import ExitStack

import numpy as np

import concourse.bass as bass
import concourse.tile as tile
from concourse import bacc, mybir
from concourse.bass_utils import run_bass_kernel_spmd

B, T, D, HS = 8, 2048, 1024, 64
N_CORES = 8
F32 = mybir.dt.float32
BF16 = mybir.dt.bfloat16

TT = 128            # t/k tile (partition dim)
NDT = D // TT       # 8 contraction chunks
NTT = T // TT       # 16 k-tiles
QS = 512            # t/q super width (matmul free dim)
NQS = T // QS       # 4 supers
VP = HS + 1         # V' width (64 + ones column)
ATTN3A = True       # interleave super-3 first half into super 2
COLPACK = True      # col-packed half-1 projection
XT0_CHUNKS = False  # chunk-granular super-0 x^T DMA
NWARM = 7           # warmup: ~3.6us continuous PE activity trips the HAM
                    # SHORT window so projection starts at 2.4 GHz


def build_graph() -> bacc.Bacc:
    nc = bacc.Bacc("TRN2", target_bir_lowering=False, debug=False)

    # host-preswizzled x^T: [4 supers x 128 partitions, 8 chunks * 512 cols]
    xts_ext = nc.dram_tensor("xts", [NQS * TT, NDT * QS], BF16,
                             kind="ExternalInput").ap()
    # host-preswizzled W: w[p, c*256+j] = wfull[c*128+p, j],
    # wfull[:, 0:128] = [Wq/8 | Wk], wfull[:, 128:256] = [Wv | 0]
    w_ext = nc.dram_tensor("wqkv", [TT, NDT * 2 * TT], BF16,
                           kind="ExternalInput").ap()
    # fp32 bias columns: col0[0:64]=bq/8, col0[64:128]=bk, col1[0:64]=bv,
    # col1[64]=1.0 (ones row for V' via the W zero-pad column)
    bcol_ext = nc.dram_tensor("bcol", [TT, 2], F32, kind="ExternalInput").ap()
    id_ext = nc.dram_tensor("ident", [TT, TT], F32, kind="ExternalInput").ap()
    idb_ext = nc.dram_tensor("identb", [TT, TT], BF16, kind="ExternalInput").ap()
    # tri-mask[k, q] = 1.0 if q >= k else 0.0
    mask_ext = nc.dram_tensor("mask", [TT, TT], BF16, kind="ExternalInput").ap()
    out_ext = nc.dram_tensor("out", [T, HS], F32, kind="ExternalOutput").ap()

    with tile.TileContext(nc) as tc, ExitStack() as ctx:
        const = ctx.enter_context(tc.tile_pool(name="const", bufs=1))
        persist = ctx.enter_context(tc.tile_pool(name="persist", bufs=1))
        vt_pool = ctx.enter_context(tc.tile_pool(name="vt", bufs=2))
        pt_pool = ctx.enter_context(tc.tile_pool(name="pt", bufs=5))
        otsb_pool = ctx.enter_context(tc.tile_pool(name="otsb", bufs=2))
        osb_pool = ctx.enter_context(tc.tile_pool(name="osb", bufs=2))
        rc_pool = ctx.enter_context(tc.tile_pool(name="rc", bufs=2))
        psum = ctx.enter_context(tc.tile_pool(name="ps", bufs=1, space="PSUM"))

        # ---- persistent SBUF ----
        xt_sb = persist.tile([TT, NQS * NDT * QS], BF16)   # all 4 supers
        w_sb = const.tile([TT, NDT * 2 * TT], BF16)
        bcol_sb = const.tile([TT, 2], F32)
        id_sb = const.tile([TT, TT], F32)
        idb_sb = const.tile([TT, TT], BF16)
        mask_sb = const.tile([TT, TT], BF16)
        warm_sb = const.tile([TT, QS], BF16)
        qt_sb = persist.tile([TT, T], BF16)     # rows 0:64 Q^T/8, 64:128 dup
        kt_sb = persist.tile([TT, (NTT // 2) * TT], BF16)  # even/odd packed
        vp_sb = persist.tile([TT, NTT * VP], BF16)         # V' per k-tile

        # ---- DMAs: x^T on the Sync HWDGE ring, weights + consts on the
        # Scalar HWDGE ring (the two rings issue in parallel) ----
        SW = NDT * QS  # 4096 cols per super
        if XT0_CHUNKS:
            for c in range(NDT):
                nc.sync.dma_start(xt_sb[:, c * QS:(c + 1) * QS],
                                  xts_ext[0:TT, c * QS:(c + 1) * QS])
        else:
            for q in range(4):
                nc.sync.dma_start(
                    xt_sb[:, q * SW // 4:(q + 1) * SW // 4],
                    xts_ext[0:TT, q * SW // 4:(q + 1) * SW // 4],
                )
        for s in range(1, NQS):
            for h in range(2):
                nc.sync.dma_start(
                    xt_sb[:, s * SW + h * SW // 2: s * SW + (h + 1) * SW // 2],
                    xts_ext[s * TT:(s + 1) * TT, h * SW // 2:(h + 1) * SW // 2],
                )
        WH = NDT * TT  # half of the w columns (chunks 0-3)
        nc.scalar.dma_start(w_sb[:, 0:WH], w_ext[:, 0:WH])
        nc.scalar.dma_start(w_sb[:, WH:2 * WH], w_ext[:, WH:2 * WH])
        nc.scalar.dma_start(bcol_sb[:], bcol_ext)
        nc.scalar.dma_start(idb_sb[:], idb_ext)
        nc.scalar.dma_start(mask_sb[:], mask_ext)
        nc.scalar.dma_start(id_sb[:], id_ext)

        # ---- PE warmup: dep-free matmuls (two alternating PSUM banks so
        # they don't drain-serialize) so the HAM activity monitor lifts the
        # PE clock to 2.4 GHz before the projection starts ----
        nc.vector.memset(warm_sb[:], 0.0)
        for i in range(NWARM):
            tag = "proj" if i % 2 == 0 else "acc"
            shape = [TT, QS] if tag == "proj" else [VP, QS]
            warm_ps = psum.tile(shape, F32, tag=tag, bufs=1, name=f"warm{i}")
            nc.tensor.matmul(
                warm_ps[0:TT if tag == "proj" else VP, :],
                warm_sb[:, 0:TT if tag == "proj" else VP],
                warm_sb[:],
                start=True, stop=True, skip_group_check=True,
            )

        def proj_ops(ts: int):
            """Emit-closures for projecting super ts (interleave units)."""
            tsl = slice(ts * QS, (ts + 1) * QS)
            ops = []
            pp_box = [None, None]

            def mk_mm(half, c):
                def _f():
                    if c == 0:
                        pp_box[half] = psum.tile([TT, QS], F32, tag="proj",
                                                 bufs=1, name=f"pp{ts}_{half}")
                    nc.tensor.matmul(
                        pp_box[half][:],
                        w_sb[:, c * 2 * TT + half * TT:c * 2 * TT + (half + 1) * TT],
                        xt_sb[:, ts * SW + c * QS:ts * SW + (c + 1) * QS],
                        start=(c == 0),
                        stop=(c == NDT - 1),
                        skip_group_check=True,
                    )
                return _f

            wtags = ["sbig", "acc", "acc2"]
            wn = [0]

            def mk_warmfill():
                def _f():
                    tag = wtags[wn[0] % 3]
                    shape = [TT, 2 * QS] if tag == "sbig" else [VP, QS]
                    wp = psum.tile(shape, F32, tag=tag,
                                   bufs=2 if tag == "sbig" else 1,
                                   name=f"wf{ts}_{wn[0]}")
                    nc.tensor.matmul(
                        wp[0:(TT if tag == "sbig" else VP), 0:QS],
                        warm_sb[:, 0:(TT if tag == "sbig" else VP)],
                        warm_sb[:],
                        start=True, stop=True, skip_group_check=True,
                    )
                    wn[0] += 1
                return _f

            for c in range(NDT):
                ops.append(mk_mm(0, c))
                if ts <= 1 and 0 < c < 6:
                    ops.append(mk_warmfill())
                    ops.append(mk_warmfill())

            def qk_evac():
                pp = pp_box[0]
                # Q^T/8 + bias -> rows 0:64, duplicated to rows 64:128
                nc.vector.tensor_scalar_add(
                    qt_sb[0:HS, tsl], pp[0:HS, :], bcol_sb[0:HS, 0:1]
                )
                nc.vector.tensor_copy(qt_sb[HS:2 * HS, tsl], qt_sb[0:HS, tsl])
                # K^T + bias, packed: k-tile 4ts+i -> pair-col u=2ts+i//2,
                # rows 0:64 for even i, 64:128 for odd i
                for i in range(4):
                    u = 2 * ts + i // 2
                    rows = slice(0, HS) if i % 2 == 0 else slice(HS, 2 * HS)
                    nc.vector.tensor_scalar_add(
                        kt_sb[rows, u * TT:(u + 1) * TT],
                        pp[HS:2 * HS, i * TT:(i + 1) * TT],
                        bcol_sb[HS:2 * HS, 0:1],
                    )
            ops.append(qk_evac)

            def mk_mm1(cpair):
                # col-packed V^T projection: chunks 2i/2i+1 run concurrently
                # in column halves of the PE array (M=64 each); only the very
                # first matmul clears the bank's has_written bits, so the odd
                # group's first write overwrites-where-clear
                def _f():
                    if cpair == 0:
                        pp_box[1] = psum.tile([TT, QS], F32, tag="proj",
                                              bufs=1, name=f"pp{ts}_1")
                        # zero the odd-group rows: HW overwrites-where-clear
                        # (start=True below clears only has_written bits),
                        # CoreSim accumulates onto this zero base
                        nc.vector.memset(pp_box[1][HS:2 * HS, :], 0.0)
                    for h in range(2):
                        c = 2 * cpair + h
                        nc.tensor.matmul(
                            pp_box[1][h * HS:(h + 1) * HS, :],
                            w_sb[:, c * 2 * TT + TT:c * 2 * TT + TT + HS],
                            xt_sb[:, ts * SW + c * QS:ts * SW + (c + 1) * QS],
                            start=(c == 0),
                            stop=(c >= NDT - 2),
                            skip_group_check=True,
                        )
                return _f

            if COLPACK:
                for cpair in range(NDT // 2):
                    ops.append(mk_mm1(cpair))
            else:
                for c in range(NDT):
                    ops.append(mk_mm(1, c))

            vt_box = [None]

            def vt_add():
                vt_box[0] = vt_pool.tile([VP, QS], F32, tag="vt", name=f"vt{ts}")
                if COLPACK:
                    # ones row for the denominator column of V'
                    nc.gpsimd.memset(vt_box[0][HS:VP, :], 1.0)
                    # vt = (V^T_even + bv) + V^T_odd -- two steps, a
                    # TensorScalarPtr may read only one PSUM operand
                    nc.vector.tensor_scalar_add(
                        vt_box[0][0:HS, :], pp_box[1][0:HS, :],
                        bcol_sb[0:HS, 1:2]
                    )
                    nc.vector.tensor_tensor(
                        vt_box[0][0:HS, :], vt_box[0][0:HS, :],
                        pp_box[1][HS:2 * HS, :], op=mybir.AluOpType.add,
                    )
                else:
                    nc.vector.tensor_scalar_add(
                        vt_box[0][0:VP, :], pp_box[1][0:VP, :],
                        bcol_sb[0:VP, 1:2]
                    )
            ops.append(vt_add)

            smv_box = [None]

            def mk_vtr(u):
                def _f():
                    if u == 0:
                        smv_box[0] = psum.tile([TT, 4 * VP], F32,
                                               tag="smo", bufs=1,
                                               name=f"smv{ts}")
                    nc.tensor.transpose(
                        smv_box[0][:, u * VP:(u + 1) * VP],
                        vt_box[0][:, u * TT:(u + 1) * TT],
                        id_sb[0:VP, 0:VP],
                    )
                return _f
            for u in range(4):
                ops.append(mk_vtr(u))

            def vp_copy():
                nc.vector.tensor_copy(
                    vp_sb[:, 4 * ts * VP:(4 * ts + 4) * VP], smv_box[0][:]
                )
            ops.append(vp_copy)
            return ops

        def mk_attn(ts, ot_get):
            """S/exp/PV emitters for super ts; PV accumulates into the
            [VP, QS] AP returned by ot_get()."""
            nkt = 4 * ts + 4
            store = {}

            def s_pair(p):
                sp = psum.tile([TT, 2 * QS], F32, tag="sbig", bufs=2,
                               name=f"sp{ts}_{p}")
                # diagonal pairs: both tiles write from the PAIR's first valid
                # column (tile B writes 128 causally-dead cols, trimmed from
                # PV) so one rectangular exp covers the pair
                c0p = TT * (2 * p - 4 * ts) if 2 * p >= 4 * ts else 0
                for h in range(2):
                    rows = slice(0, HS) if h == 0 else slice(HS, 2 * HS)
                    nc.tensor.matmul(
                        sp[:, h * QS + c0p:(h + 1) * QS],
                        kt_sb[rows, p * TT:(p + 1) * TT],
                        qt_sb[rows, ts * QS + c0p:(ts + 1) * QS],
                        start=True,
                        stop=True,
                        skip_group_check=True,
                    )
                store[("s", p)] = sp

            def do_exp(p):
                sp = store.pop(("s", p))
                ptile = pt_pool.tile([TT, 2 * QS], BF16, tag="pt",
                                     name=f"pt{ts}_{p}")
                if 2 * p >= 4 * ts:
                    # diagonal pair: one strided activation over both tiles'
                    # written ranges
                    c0 = TT * (2 * p - 4 * ts)
                    sp3 = sp[:].rearrange("k (h q) -> k h q", h=2)
                    pt3 = ptile[:].rearrange("k (h q) -> k h q", h=2)
                    nc.scalar.activation(
                        pt3[:, :, c0:QS], sp3[:, :, c0:QS],
                        mybir.ActivationFunctionType.Exp,
                    )
                else:
                    # off-diagonal pair: one activation over both tiles
                    nc.scalar.activation(
                        ptile[:], sp[:], mybir.ActivationFunctionType.Exp
                    )
                for h in range(2):
                    jj = 2 * p + h
                    if jj >= 4 * ts:
                        # zero P^T[k, c] where c < k on the diagonal band
                        # (GpSimd is otherwise idle; keeps the DVE queue out
                        # of the S->exp->mask->PV chain)
                        b0 = h * QS + TT * (jj - 4 * ts)
                        nc.gpsimd.affine_select(
                            out=ptile[:, b0:b0 + TT],
                            in_=ptile[:, b0:b0 + TT],
                            compare_op=mybir.AluOpType.is_ge,
                            fill=0.0,
                            base=0,
                            channel_multiplier=-1,
                            pattern=[[1, TT]],
                        )
                store[("p", p)] = ptile

            def pv(p, is_first, is_last):
                ptile = store.pop(("p", p))
                ot = ot_get()
                for h in range(2):
                    jj = 2 * p + h
                    c0 = TT * (jj - 4 * ts) if jj >= 4 * ts else 0
                    nc.tensor.matmul(
                        ot[:, c0:QS],
                        vp_sb[:, jj * VP:(jj + 1) * VP],
                        ptile[:, h * QS + c0:(h + 1) * QS],
                        start=(is_first and h == 0),
                        stop=(is_last and h == 1),
                        skip_group_check=True,
                    )

            return s_pair, do_exp, pv

        def emit_super(ts, filler, head=None, pending_ep=None, ot_get=None,
                       p_lo=0, split_last_ep=False):
            """Attention pairs [p_lo, npair) of super ts; `filler` ops are
            interleaved ahead of each PV (so the PE queue head never blocks
            on the exp chain), `head` is emitted in full before the first
            PV, and the previous super's epilogue (`pending_ep`) is emitted
            into this super's pipeline-fill window. Returns this super's
            epilogue closure."""
            npair = (4 * ts + 4) // 2
            if ot_get is None:
                ot = psum.tile([VP, QS], F32, tag="acc", bufs=1,
                               name=f"ot{ts}")
                ot_get = lambda: ot[:]
            s_pair, do_exp, pv = mk_attn(ts, ot_get)
            fill_i = [0]

            def emit_fill(frac_done):
                tgt = int(round(frac_done * len(filler)))
                while fill_i[0] < tgt:
                    filler[fill_i[0]]()
                    fill_i[0] += 1

            if p_lo == 0:
                # diagonal pairs (longest exp chains: 2 activations +
                # affine_select) first, so they overlap the off-diagonal
                # pipeline instead of draining at the super boundary; the
                # first-emitted PV (jj=4ts) is full-width, so start=True
                # covers the whole bank
                seq = [2 * ts, 2 * ts + 1] + list(range(0, 2 * ts))
            else:
                seq = list(range(p_lo, npair))
            s_pair(seq[0])
            for k, p in enumerate(seq):
                if k + 1 < len(seq):
                    s_pair(seq[k + 1])
                if k == 0 and pending_ep is not None:
                    pending_ep()
                do_exp(p)
                if k == 0 and head:
                    for op in head:
                        op()
                emit_fill((k + 1) / len(seq))
                if split_last_ep and k == len(seq) - 1:
                    # subtiles 0-1 are final once the previous PV is done;
                    # flush them while exp of the last pair runs
                    mk_epilogue(ts, ot_get(), 0, 2)
                pv(p, is_first=(p_lo == 0 and k == 0),
                   is_last=(p == npair - 1 and ts == NQS - 1))

            if split_last_ep:
                return lambda: mk_epilogue(ts, ot_get(), 2, 4)
            return lambda: mk_epilogue(ts, ot_get(), 0, 4)

        def mk_epilogue(ts, ot, u0, u1):
            # -- epilogue (fp32): normalize + transpose back + store --
            nu = u1 - u0
            ot_sb = otsb_pool.tile([VP, QS // 4 * nu], F32, tag="otsb",
                                   name=f"ot_sb{ts}_{u0}")
            nc.vector.tensor_copy(ot_sb[:], ot[:, u0 * TT:u1 * TT])
            smo = psum.tile([TT, 4 * VP], F32, tag="smo", bufs=1,
                            name=f"smo{ts}_{u0}")
            for i in range(nu):
                nc.tensor.transpose(
                    smo[:, i * VP:(i + 1) * VP],
                    ot_sb[:, i * TT:(i + 1) * TT],
                    id_sb[0:VP, 0:VP],
                )
            o_sb = osb_pool.tile([TT, nu * HS], F32, tag="osb",
                                 name=f"o_sb{ts}_{u0}")
            rc = rc_pool.tile([TT, nu], F32, tag="rc", name=f"rc{ts}_{u0}")
            smo3 = smo[:, 0:nu * VP].rearrange("p (u v) -> p u v", u=nu)
            rc3 = rc[:].rearrange("p (u v) -> p u v", v=1)
            nc.vector.reciprocal(rc3[:, :, :], smo3[:, :, HS:HS + 1])
            for i in range(nu):
                nc.vector.tensor_scalar_mul(
                    o_sb[:, i * HS:(i + 1) * HS],
                    smo[:, i * VP:i * VP + HS],
                    rc[:, i:i + 1],
                )
            rows = slice(ts * QS + u0 * TT, ts * QS + u1 * TT)
            nc.sync.dma_start(
                out_ext[rows, :].rearrange("(u p) h -> p u h", p=TT),
                o_sb[:].rearrange("p (u h) -> p u h", u=nu),
            )

        # super-0 projection: first half + QK evacuation up front so S can
        # start; the rest rides inside attention(0) ahead of its first PV.
        # Each super's epilogue is deferred into the next super's window.
        ops0 = proj_ops(0)
        n0 = len(ops0) - 5  # through vt_add (vtrans+copy stay as head)
        for op in ops0[:n0]:
            op()
        # wavefront one level deeper: super-1's first two pairs (k-tiles 0-3
        # need only super-0 K/V) ride inside attention(0) on the acc2 bank,
        # filling the ScalarE hole after super-0's exps; super-2's
        # accumulator then takes the acc bank right after epilogue(0) retires
        # it, and super-2's first four pairs ride inside attention(1)
        ot1_box = [None]

        def alloc_ot1():
            ot1_box[0] = psum.tile([VP, QS], F32, tag="acc2", bufs=1,
                                   name="ot1")

        ot1_get = lambda: ot1_box[0][:]
        s1, e1, v1 = mk_attn(1, ot1_get)
        ops1a = [alloc_ot1]
        for p in range(2):
            ops1a += [lambda p=p: s1(p), lambda p=p: e1(p),
                      lambda p=p: v1(p, p == 0, False)]
        ep0 = emit_super(0, proj_ops(1) + ops1a, head=ops0[n0:])
        ot2_box = [None]

        def alloc_ot2():
            ot2_box[0] = psum.tile([VP, QS], F32, tag="acc", bufs=1,
                                   name="ot2")

        ot2_get = lambda: ot2_box[0][:]
        s2, e2, v2 = mk_attn(2, ot2_get)
        ops2a = [alloc_ot2]
        for p in range(4):
            ops2a += [lambda p=p: s2(p), lambda p=p: e2(p),
                      lambda p=p: v2(p, p == 0, False)]
        ep1 = emit_super(1, proj_ops(2) + ops2a, pending_ep=ep0,
                         ot_get=ot1_get, p_lo=2)
        # super 3's first half (k-tiles 0-7 need only supers 0-1 K/V) rides
        # inside the PE-bound attention of super 2, using ScalarE slack there;
        # its accumulator comes lazily from the proj psum tag (free after
        # proj(3)'s last evacuation)
        if ATTN3A:
            ot3_box = [None]

            def alloc_ot3():
                ot3_box[0] = psum.tile([TT, QS], F32, tag="proj", bufs=1,
                                       name="ot3")

            ot3_get = lambda: ot3_box[0][0:VP, :]
            s3, e3, v3 = mk_attn(3, ot3_get)
            ops3a = [alloc_ot3]
            for p in range(4):
                ops3a += [lambda p=p: s3(p), lambda p=p: e3(p),
                          lambda p=p: v3(p, p == 0, False)]
            ep2 = emit_super(2, proj_ops(3) + ops3a, pending_ep=ep1,
                             ot_get=ot2_get, p_lo=4)
            ep3 = emit_super(3, [], pending_ep=ep2, ot_get=ot3_get, p_lo=4,
                             split_last_ep=True)
        else:
            ep2 = emit_super(2, proj_ops(3), pending_ep=ep1)
            ep3 = emit_super(3, [], pending_ep=ep2, split_last_ep=True)
        ep3()

    nc.compile()
    return nc


def make_inputs(x_b, Wq, bq, Wk, bk, Wv, bv):
    """Host-side prep for one core's in_map (x_b: [T, D] fp32)."""
    import ml_dtypes

    bf = ml_dtypes.bfloat16
    scale = 1.0 / np.sqrt(np.float32(HS))
    w = np.zeros((D, 2 * TT), dtype=np.float32)
    w[:, 0:HS] = Wq * scale
    w[:, HS:2 * HS] = Wk
    w[:, 2 * HS:3 * HS] = Wv
    # swizzle: w_sb[p, c*256+j] = w[c*128+p, j]
    wsw = np.ascontiguousarray(
        w.reshape(NDT, TT, 2 * TT).transpose(1, 0, 2).reshape(TT, NDT * 2 * TT)
    )
    bcol = np.zeros((TT, 2), dtype=np.float32)
    bcol[0:HS, 0] = bq * scale
    bcol[HS:2 * HS, 0] = bk
    bcol[0:HS, 1] = bv
    bcol[HS, 1] = 1.0
    # xts[s*128+p, c*512+q] = x_b[s*512+q, c*128+p]
    xts = np.ascontiguousarray(
        x_b.reshape(NQS, QS, NDT, TT).transpose(0, 3, 2, 1)
        .reshape(NQS * TT, NDT * QS)
    ).astype(bf)
    mask = np.triu(np.ones((TT, TT), dtype=bf))
    return {
        "xts": xts,
        "wqkv": wsw.astype(bf),
        "bcol": bcol,
        "ident": np.eye(TT, dtype=np.float32),
        "identb": np.eye(TT, dtype=bf),
        "mask": mask,
    }


_NC_CACHE = None


def _get_nc():
    global _NC_CACHE
    if _NC_CACHE is None:
        _NC_CACHE = build_graph()
    return _NC_CACHE


def kernel(x, Wq, bq, Wk, bk, Wv, bv):
    x = np.asarray(x, dtype=np.float32)
    args = [np.asarray(a, dtype=np.float32) for a in (Wq, bq, Wk, bk, Wv, bv)]
    nc = _get_nc()
    in_maps = [make_inputs(x[b], *args) for b in range(N_CORES)]
    trace = os.environ.get("BASS_ATTN_TRACE", "0") == "1"
    res = run_bass_kernel_spmd(
        nc, in_maps, core_ids=list(range(N_CORES)), trace=trace
    )
    if trace:
        print(
            f"HW exec time: {res.exec_time_ns} ns "
            f"(mean {res.mean_exec_time_ns}, max core {res.max_exec_time_core_id})"
        )
    out = np.stack([res.results[b]["out"] for b in range(N_CORES)], axis=0)
    return out


# revision 38
# speedup vs baseline: 1.1021x; 1.1021x over previous
"""Causal single-head attention (B=8, T=2048, D=1024, HS=64) on 8 TRN2 NeuronCores.

Sharding: data-parallel over batch -- core b computes batch b end-to-end.
No collectives; outputs are concatenated on the host.

Compute path is bf16 (operands) with fp32 PSUM accumulation; softmax
denominator/normalization stays fp32. The host casts x/W to bf16 and
pre-swizzles both into their exact SBUF layouts so every DMA is a large
contiguous transfer; x^T rides the Sync HWDGE ring, weights/consts ride the
Scalar HWDGE ring in parallel.

Per-core pipeline, four 512-col t-supers, software-pipelined across supers:
  ~10 dep-free warmup matmuls keep the PE busy while the first DMAs land so
  the HAM activity monitor lifts the PE clock to 2.4 GHz before projection
  projection, W-stationary:  QKV^T[:,t] = W^T x^T
    half 0: [Wq/8 | Wk] (1/sqrt(HS) folded into Wq/bq)
    half 1: Wv col-packed -- chunk pairs run concurrently in the two column
    halves of the PE array (M=64 each, one shared PSUM bank; only the very
    first matmul clears has_written, so the odd group overwrites-where-clear
    onto a zeroed base)
  DVE evacuations: Q^T duplicated to partitions 64:128, K^T packed even/odd
  k-tile into partitions 0:64/64:128 (so the row-tiled S pair shares the
  array), V^T summed+biased -> PE-transpose -> V' [k, 65] (ones row from a
  GpSimd memset)
  attention in k-tile PAIRS: both S matmuls of a pair occupy disjoint row
  groups (contraction is 64) and run concurrently when the PE queue has
  backlog; causally-invalid columns are trimmed from S / exp / PV; one exp
  per pair (diagonal pairs write 128 causally-dead columns in the second S
  matmul so a single strided activation covers both tiles); the diagonal
  triangle is zeroed by GpSimd affine_select (off the DVE/ScalarE queues):
    S^T[k,q] = K-pair @ Q^T-super       (PSUM fp32, two banks, trimmed)
    P^T = exp(S^T)                      (ScalarE; logits ~N(0,1), no max sub)
    outT[65,q] += V'[k,65]^T @ P^T      (PSUM fp32 accumulate; row 64=denom)
  projection of super ts+1 is interleaved AHEAD of each PV in the pair loop
  of super ts, so the PE FIFO head never blocks on the exp chain; each
  super's epilogue is deferred into the next super's pipeline-fill window;
  wavefront: super 2's first four pairs ride inside attention(1) and super
  3's first half rides inside attention(2) (each needs only earlier supers'
  K/V), soaking up ScalarE idle between exp bursts -- super 3 accumulates
  into the retired projection PSUM bank; dep-free warm matmuls are woven
  between the DMA-paced projection chunks of supers 0-1 so the HAM never
  re-throttles; the last super's epilogue is split so subtiles 0-1 flush
  during the final exp.
  epilogue (fp32): PE-transpose outT back, DVE reciprocal + scale, DMA out.
"""

import sys

if "/opt/trn_rl_repo" not in sys.path:
    sys.path.insert(0, "/opt/trn_rl_repo")

import os
from contextlib import ExitStack

import numpy as np

import concourse.bass as bass
import concourse.tile as tile
from concourse import bacc, mybir
from concourse.bass_utils import run_bass_kernel_spmd

B, T, D, HS = 8, 2048, 1024, 64
N_CORES = 8
F32 = mybir.dt.float32
BF16 = mybir.dt.bfloat16

TT = 128            # t/k tile (partition dim)
NDT = D // TT       # 8 contraction chunks
NTT = T // TT       # 16 k-tiles
QS = 512            # t/q super width (matmul free dim)
NQS = T // QS       # 4 supers
VP = HS + 1         # V' width (64 + ones column)
ATTN3A = True       # interleave super-3 first half into super 2
COLPACK = True      # col-packed half-1 projection
XT0_CHUNKS = False  # chunk-granular super-0 x^T DMA
NWARM = 10          # warmup: ~5us continuous PE activity trips the HAM
                    # SHORT window so projection starts at 2.4 GHz


def build_graph() -> bacc.Bacc:
    nc = bacc.Bacc("TRN2", target_bir_lowering=False, debug=False)

    # host-preswizzled x^T: [4 supers x 128 partitions, 8 chunks * 512 cols]
    xts_ext = nc.dram_tensor("xts", [NQS * TT, NDT * QS], BF16,
                             kind="ExternalInput").ap()
    # host-preswizzled W: w[p, c*256+j] = wfull[c*128+p, j],
    # wfull[:, 0:128] = [Wq/8 | Wk], wfull[:, 128:256] = [Wv | 0]
    w_ext = nc.dram_tensor("wqkv", [TT, NDT * 2 * TT], BF16,
                           kind="ExternalInput").ap()
    # fp32 bias columns: col0[0:64]=bq/8, col0[64:128]=bk, col1[0:64]=bv,
    # col1[64]=1.0 (ones row for V' via the W zero-pad column)
    bcol_ext = nc.dram_tensor("bcol", [TT, 2], F32, kind="ExternalInput").ap()
    id_ext = nc.dram_tensor("ident", [TT, TT], F32, kind="ExternalInput").ap()
    idb_ext = nc.dram_tensor("identb", [TT, TT], BF16, kind="ExternalInput").ap()
    # tri-mask[k, q] = 1.0 if q >= k else 0.0
    mask_ext = nc.dram_tensor("mask", [TT, TT], BF16, kind="ExternalInput").ap()
    out_ext = nc.dram_tensor("out", [T, HS], F32, kind="ExternalOutput").ap()

    with tile.TileContext(nc) as tc, ExitStack() as ctx:
        const = ctx.enter_context(tc.tile_pool(name="const", bufs=1))
        persist = ctx.enter_context(tc.tile_pool(name="persist", bufs=1))
        vt_pool = ctx.enter_context(tc.tile_pool(name="vt", bufs=2))
        pt_pool = ctx.enter_context(tc.tile_pool(name="pt", bufs=5))
        otsb_pool = ctx.enter_context(tc.tile_pool(name="otsb", bufs=2))
        osb_pool = ctx.enter_context(tc.tile_pool(name="osb", bufs=2))
        rc_pool = ctx.enter_context(tc.tile_pool(name="rc", bufs=2))
        psum = ctx.enter_context(tc.tile_pool(name="ps", bufs=1, space="PSUM"))

        # ---- persistent SBUF ----
        xt_sb = persist.tile([TT, NQS * NDT * QS], BF16)   # all 4 supers
        w_sb = const.tile([TT, NDT * 2 * TT], BF16)
        bcol_sb = const.tile([TT, 2], F32)
        id_sb = const.tile([TT, TT], F32)
        idb_sb = const.tile([TT, TT], BF16)
        mask_sb = const.tile([TT, TT], BF16)
        warm_sb = const.tile([TT, QS], BF16)
        qt_sb = persist.tile([TT, T], BF16)     # rows 0:64 Q^T/8, 64:128 dup
        kt_sb = persist.tile([TT, (NTT // 2) * TT], BF16)  # even/odd packed
        vp_sb = persist.tile([TT, NTT * VP], BF16)         # V' per k-tile

        # ---- DMAs: x^T on the Sync HWDGE ring, weights + consts on the
        # Scalar HWDGE ring (the two rings issue in parallel) ----
        SW = NDT * QS  # 4096 cols per super
        if XT0_CHUNKS:
            for c in range(NDT):
                nc.sync.dma_start(xt_sb[:, c * QS:(c + 1) * QS],
                                  xts_ext[0:TT, c * QS:(c + 1) * QS])
        else:
            for q in range(4):
                nc.sync.dma_start(
                    xt_sb[:, q * SW // 4:(q + 1) * SW // 4],
                    xts_ext[0:TT, q * SW // 4:(q + 1) * SW // 4],
                )
        for s in range(1, NQS):
            for h in range(2):
                nc.sync.dma_start(
                    xt_sb[:, s * SW + h * SW // 2: s * SW + (h + 1) * SW // 2],
                    xts_ext[s * TT:(s + 1) * TT, h * SW // 2:(h + 1) * SW // 2],
                )
        WH = NDT * TT  # half of the w columns (chunks 0-3)
        nc.scalar.dma_start(w_sb[:, 0:WH], w_ext[:, 0:WH])
        nc.scalar.dma_start(w_sb[:, WH:2 * WH], w_ext[:, WH:2 * WH])
        nc.scalar.dma_start(bcol_sb[:], bcol_ext)
        nc.scalar.dma_start(idb_sb[:], idb_ext)
        nc.scalar.dma_start(mask_sb[:], mask_ext)
        nc.scalar.dma_start(id_sb[:], id_ext)

        # ---- PE warmup: dep-free matmuls (two alternating PSUM banks so
        # they don't drain-serialize) so the HAM activity monitor lifts the
        # PE clock to 2.4 GHz before the projection starts ----
        nc.vector.memset(warm_sb[:], 0.0)
        for i in range(NWARM):
            tag = "proj" if i % 2 == 0 else "acc"
            shape = [TT, QS] if tag == "proj" else [VP, QS]
            warm_ps = psum.tile(shape, F32, tag=tag, bufs=1, name=f"warm{i}")
            nc.tensor.matmul(
                warm_ps[0:TT if tag == "proj" else VP, :],
                warm_sb[:, 0:TT if tag == "proj" else VP],
                warm_sb[:],
                start=True, stop=True, skip_group_check=True,
            )

        def proj_ops(ts: int):
            """Emit-closures for projecting super ts (interleave units)."""
            tsl = slice(ts * QS, (ts + 1) * QS)
            ops = []
            pp_box = [None, None]

            def mk_mm(half, c):
                def _f():
                    if c == 0:
                        pp_box[half] = psum.tile([TT, QS], F32, tag="proj",
                                                 bufs=1, name=f"pp{ts}_{half}")
                    nc.tensor.matmul(
                        pp_box[half][:],
                        w_sb[:, c * 2 * TT + half * TT:c * 2 * TT + (half + 1) * TT],
                        xt_sb[:, ts * SW + c * QS:ts * SW + (c + 1) * QS],
                        start=(c == 0),
                        stop=(c == NDT - 1),
                        skip_group_check=True,
                    )
                return _f

            wtags = ["sbig", "acc", "acc2"]
            wn = [0]

            def mk_warmfill():
                def _f():
                    tag = wtags[wn[0] % 3]
                    shape = [TT, 2 * QS] if tag == "sbig" else [VP, QS]
                    wp = psum.tile(shape, F32, tag=tag,
                                   bufs=2 if tag == "sbig" else 1,
                                   name=f"wf{ts}_{wn[0]}")
                    nc.tensor.matmul(
                        wp[0:(TT if tag == "sbig" else VP), 0:QS],
                        warm_sb[:, 0:(TT if tag == "sbig" else VP)],
                        warm_sb[:],
                        start=True, stop=True, skip_group_check=True,
                    )
                    wn[0] += 1
                return _f

            for c in range(NDT):
                ops.append(mk_mm(0, c))
                if ts <= 1 and 0 < c < 6:
                    ops.append(mk_warmfill())
                    ops.append(mk_warmfill())

            def qk_evac():
                pp = pp_box[0]
                # Q^T/8 + bias -> rows 0:64, duplicated to rows 64:128
                nc.vector.tensor_scalar_add(
                    qt_sb[0:HS, tsl], pp[0:HS, :], bcol_sb[0:HS, 0:1]
                )
                nc.vector.tensor_copy(qt_sb[HS:2 * HS, tsl], qt_sb[0:HS, tsl])
                # K^T + bias, packed: k-tile 4ts+i -> pair-col u=2ts+i//2,
                # rows 0:64 for even i, 64:128 for odd i
                for i in range(4):
                    u = 2 * ts + i // 2
                    rows = slice(0, HS) if i % 2 == 0 else slice(HS, 2 * HS)
                    nc.vector.tensor_scalar_add(
                        kt_sb[rows, u * TT:(u + 1) * TT],
                        pp[HS:2 * HS, i * TT:(i + 1) * TT],
                        bcol_sb[HS:2 * HS, 0:1],
                    )
            ops.append(qk_evac)

            def mk_mm1(cpair):
                # col-packed V^T projection: chunks 2i/2i+1 run concurrently
                # in column halves of the PE array (M=64 each); only the very
                # first matmul clears the bank's has_written bits, so the odd
                # group's first write overwrites-where-clear
                def _f():
                    if cpair == 0:
                        pp_box[1] = psum.tile([TT, QS], F32, tag="proj",
                                              bufs=1, name=f"pp{ts}_1")
                        # zero the odd-group rows: HW overwrites-where-clear
                        # (start=True below clears only has_written bits),
                        # CoreSim accumulates onto this zero base
                        nc.vector.memset(pp_box[1][HS:2 * HS, :], 0.0)
                    for h in range(2):
                        c = 2 * cpair + h
                        nc.tensor.matmul(
                            pp_box[1][h * HS:(h + 1) * HS, :],
                            w_sb[:, c * 2 * TT + TT:c * 2 * TT + TT + HS],
                            xt_sb[:, ts * SW + c * QS:ts * SW + (c + 1) * QS],
                            start=(c == 0),
                            stop=(c >= NDT - 2),
                            skip_group_check=True,
                        )
                return _f

            if COLPACK:
                for cpair in range(NDT // 2):
                    ops.append(mk_mm1(cpair))
            else:
                for c in range(NDT):
                    ops.append(mk_mm(1, c))

            vt_box = [None]

            def vt_add():
                vt_box[0] = vt_pool.tile([VP, QS], F32, tag="vt", name=f"vt{ts}")
                if COLPACK:
                    # ones row for the denominator column of V'
                    nc.gpsimd.memset(vt_box[0][HS:VP, :], 1.0)
                    # vt = (V^T_even + bv) + V^T_odd -- two steps, a
                    # TensorScalarPtr may read only one PSUM operand
                    nc.vector.tensor_scalar_add(
                        vt_box[0][0:HS, :], pp_box[1][0:HS, :],
                        bcol_sb[0:HS, 1:2]
                    )
                    nc.vector.tensor_tensor(
                        vt_box[0][0:HS, :], vt_box[0][0:HS, :],
                        pp_box[1][HS:2 * HS, :], op=mybir.AluOpType.add,
                    )
                else:
                    nc.vector.tensor_scalar_add(
                        vt_box[0][0:VP, :], pp_box[1][0:VP, :],
                        bcol_sb[0:VP, 1:2]
                    )
            ops.append(vt_add)

            smv_box = [None]

            def mk_vtr(u):
                def _f():
                    if u == 0:
                        smv_box[0] = psum.tile([TT, 4 * VP], F32,
                                               tag="smo", bufs=1,
                                               name=f"smv{ts}")
                    nc.tensor.transpose(
                        smv_box[0][:, u * VP:(u + 1) * VP],
                        vt_box[0][:, u * TT:(u + 1) * TT],
                        id_sb[0:VP, 0:VP],
                    )
                return _f
            for u in range(4):
                ops.append(mk_vtr(u))

            def vp_copy():
                nc.vector.tensor_copy(
                    vp_sb[:, 4 * ts * VP:(4 * ts + 4) * VP], smv_box[0][:]
                )
            ops.append(vp_copy)
            return ops

        def mk_attn(ts, ot_get):
            """S/exp/PV emitters for super ts; PV accumulates into the
            [VP, QS] AP returned by ot_get()."""
            nkt = 4 * ts + 4
            store = {}

            def s_pair(p):
                sp = psum.tile([TT, 2 * QS], F32, tag="sbig", bufs=2,
                               name=f"sp{ts}_{p}")
                # diagonal pairs: both tiles write from the PAIR's first valid
                # column (tile B writes 128 causally-dead cols, trimmed from
                # PV) so one rectangular exp covers the pair
                c0p = TT * (2 * p - 4 * ts) if 2 * p >= 4 * ts else 0
                for h in range(2):
                    rows = slice(0, HS) if h == 0 else slice(HS, 2 * HS)
                    nc.tensor.matmul(
                        sp[:, h * QS + c0p:(h + 1) * QS],
                        kt_sb[rows, p * TT:(p + 1) * TT],
                        qt_sb[rows, ts * QS + c0p:(ts + 1) * QS],
                        start=True,
                        stop=True,
                        skip_group_check=True,
                    )
                store[("s", p)] = sp

            def do_exp(p):
                sp = store.pop(("s", p))
                ptile = pt_pool.tile([TT, 2 * QS], BF16, tag="pt",
                                     name=f"pt{ts}_{p}")
                if 2 * p >= 4 * ts:
                    # diagonal pair: one strided activation over both tiles'
                    # written ranges
                    c0 = TT * (2 * p - 4 * ts)
                    sp3 = sp[:].rearrange("k (h q) -> k h q", h=2)
                    pt3 = ptile[:].rearrange("k (h q) -> k h q", h=2)
                    nc.scalar.activation(
                        pt3[:, :, c0:QS], sp3[:, :, c0:QS],
                        mybir.ActivationFunctionType.Exp,
                    )
                else:
                    # off-diagonal pair: one activation over both tiles
                    nc.scalar.activation(
                        ptile[:], sp[:], mybir.ActivationFunctionType.Exp
                    )
                for h in range(2):
                    jj = 2 * p + h
                    if jj >= 4 * ts:
                        # zero P^T[k, c] where c < k on the diagonal band
                        # (GpSimd is otherwise idle; keeps the DVE queue out
                        # of the S->exp->mask->PV chain)
                        b0 = h * QS + TT * (jj - 4 * ts)
                        nc.gpsimd.affine_select(
                            out=ptile[:, b0:b0 + TT],
                            in_=ptile[:, b0:b0 + TT],
                            compare_op=mybir.AluOpType.is_ge,
                            fill=0.0,
                            base=0,
                            channel_multiplier=-1,
                            pattern=[[1, TT]],
                        )
                store[("p", p)] = ptile

            def pv(p, is_first, is_last):
                ptile = store.pop(("p", p))
                ot = ot_get()
                for h in range(2):
                    jj = 2 * p + h
                    c0 = TT * (jj - 4 * ts) if jj >= 4 * ts else 0
                    nc.tensor.matmul(
                        ot[:, c0:QS],
                        vp_sb[:, jj * VP:(jj + 1) * VP],
                        ptile[:, h * QS + c0:(h + 1) * QS],
                        start=(is_first and h == 0),
                        stop=(is_last and h == 1),
                        skip_group_check=True,
                    )

            return s_pair, do_exp, pv

        def emit_super(ts, filler, head=None, pending_ep=None, ot_get=None,
                       p_lo=0, split_last_ep=False):
            """Attention pairs [p_lo, npair) of super ts; `filler` ops are
            interleaved ahead of each PV (so the PE queue head never blocks
            on the exp chain), `head` is emitted in full before the first
            PV, and the previous super's epilogue (`pending_ep`) is emitted
            into this super's pipeline-fill window. Returns this super's
            epilogue closure."""
            npair = (4 * ts + 4) // 2
            if ot_get is None:
                ot = psum.tile([VP, QS], F32, tag="acc", bufs=1,
                               name=f"ot{ts}")
                ot_get = lambda: ot[:]
            s_pair, do_exp, pv = mk_attn(ts, ot_get)
            fill_i = [0]

            def emit_fill(frac_done):
                tgt = int(round(frac_done * len(filler)))
                while fill_i[0] < tgt:
                    filler[fill_i[0]]()
                    fill_i[0] += 1

            if p_lo == 0:
                # diagonal pairs (longest exp chains: 2 activations +
                # affine_select) first, so they overlap the off-diagonal
                # pipeline instead of draining at the super boundary; the
                # first-emitted PV (jj=4ts) is full-width, so start=True
                # covers the whole bank
                seq = [2 * ts, 2 * ts + 1] + list(range(0, 2 * ts))
            else:
                seq = list(range(p_lo, npair))
            s_pair(seq[0])
            for k, p in enumerate(seq):
                if k + 1 < len(seq):
                    s_pair(seq[k + 1])
                if k == 0 and pending_ep is not None:
                    pending_ep()
                do_exp(p)
                if k == 0 and head:
                    for op in head:
                        op()
                emit_fill((k + 1) / len(seq))
                if split_last_ep and k == len(seq) - 1:
                    # subtiles 0-1 are final once the previous PV is done;
                    # flush them while exp of the last pair runs
                    mk_epilogue(ts, ot_get(), 0, 2)
                pv(p, is_first=(p_lo == 0 and k == 0),
                   is_last=(p == npair - 1 and ts == NQS - 1))

            if split_last_ep:
                return lambda: mk_epilogue(ts, ot_get(), 2, 4)
            return lambda: mk_epilogue(ts, ot_get(), 0, 4)

        def mk_epilogue(ts, ot, u0, u1):
            # -- epilogue (fp32): normalize + transpose back + store --
            nu = u1 - u0
            ot_sb = otsb_pool.tile([VP, QS // 4 * nu], F32, tag="otsb",
                                   name=f"ot_sb{ts}_{u0}")
            nc.vector.tensor_copy(ot_sb[:], ot[:, u0 * TT:u1 * TT])
            smo = psum.tile([TT, 4 * VP], F32, tag="smo", bufs=1,
                            name=f"smo{ts}_{u0}")
            for i in range(nu):
                nc.tensor.transpose(
                    smo[:, i * VP:(i + 1) * VP],
                    ot_sb[:, i * TT:(i + 1) * TT],
                    id_sb[0:VP, 0:VP],
                )
            o_sb = osb_pool.tile([TT, nu * HS], F32, tag="osb",
                                 name=f"o_sb{ts}_{u0}")
            rc = rc_pool.tile([TT, nu], F32, tag="rc", name=f"rc{ts}_{u0}")
            smo3 = smo[:, 0:nu * VP].rearrange("p (u v) -> p u v", u=nu)
            rc3 = rc[:].rearrange("p (u v) -> p u v", v=1)
            nc.vector.reciprocal(rc3[:, :, :], smo3[:, :, HS:HS + 1])
            for i in range(nu):
                nc.vector.tensor_scalar_mul(
                    o_sb[:, i * HS:(i + 1) * HS],
                    smo[:, i * VP:i * VP + HS],
                    rc[:, i:i + 1],
                )
            rows = slice(ts * QS + u0 * TT, ts * QS + u1 * TT)
            nc.sync.dma_start(
                out_ext[rows, :].rearrange("(u p) h -> p u h", p=TT),
                o_sb[:].rearrange("p (u h) -> p u h", u=nu),
            )

        # super-0 projection: first half + QK evacuation up front so S can
        # start; the rest rides inside attention(0) ahead of its first PV.
        # Each super's epilogue is deferred into the next super's window.
        ops0 = proj_ops(0)
        n0 = len(ops0) - 5  # through vt_add (vtrans+copy stay as head)
        for op in ops0[:n0]:
            op()
        # wavefront one level deeper: super-1's first two pairs (k-tiles 0-3
        # need only super-0 K/V) ride inside attention(0) on the acc2 bank,
        # filling the ScalarE hole after super-0's exps; super-2's
        # accumulator then takes the acc bank right after epilogue(0) retires
        # it, and super-2's first four pairs ride inside attention(1)
        ot1_box = [None]

        def alloc_ot1():
            ot1_box[0] = psum.tile([VP, QS], F32, tag="acc2", bufs=1,
                                   name="ot1")

        ot1_get = lambda: ot1_box[0][:]
        s1, e1, v1 = mk_attn(1, ot1_get)
        ops1a = [alloc_ot1]
        for p in range(2):
            ops1a += [lambda p=p: s1(p), lambda p=p: e1(p),
                      lambda p=p: v1(p, p == 0, False)]
        ep0 = emit_super(0, proj_ops(1) + ops1a, head=ops0[n0:])
        ot2_box = [None]

        def alloc_ot2():
            ot2_box[0] = psum.tile([VP, QS], F32, tag="acc", bufs=1,
                                   name="ot2")

        ot2_get = lambda: ot2_box[0][:]
        s2, e2, v2 = mk_attn(2, ot2_get)
        ops2a = [alloc_ot2]
        for p in range(4):
            ops2a += [lambda p=p: s2(p), lambda p=p: e2(p),
                      lambda p=p: v2(p, p == 0, False)]
        ep1 = emit_super(1, proj_ops(2) + ops2a, pending_ep=ep0,
                         ot_get=ot1_get, p_lo=2)
        # super 3's first half (k-tiles 0-7 need only supers 0-1 K/V) rides
        # inside the PE-bound attention of super 2, using ScalarE slack there;
        # its accumulator comes lazily from the proj psum tag (free after
        # proj(3)'s last evacuation)
        if ATTN3A:
            ot3_box = [None]

            def alloc_ot3():
                ot3_box[0] = psum.tile([TT, QS], F32, tag="proj", bufs=1,
                                       name="ot3")

            ot3_get = lambda: ot3_box[0][0:VP, :]
            s3, e3, v3 = mk_attn(3, ot3_get)
            ops3a = [alloc_ot3]
            for p in range(4):
                ops3a += [lambda p=p: s3(p), lambda p=p: e3(p),
                          lambda p=p: v3(p, p == 0, False)]
            ep2 = emit_super(2, proj_ops(3) + ops3a, pending_ep=ep1,
                             ot_get=ot2_get, p_lo=4)
            ep3 = emit_super(3, [], pending_ep=ep2, ot_get=ot3_get, p_lo=4,
                             split_last_ep=True)
        else:
            ep2 = emit_super(2, proj_ops(3), pending_ep=ep1)
            ep3 = emit_super(3, [], pending_ep=ep2, split_last_ep=True)
        ep3()

    nc.compile()
    return nc


def make_inputs(x_b, Wq, bq, Wk, bk, Wv, bv):
    """Host-side prep for one core's in_map (x_b: [T, D] fp32)."""
    import ml_dtypes

    bf = ml_dtypes.bfloat16
    scale = 1.0 / np.sqrt(np.float32(HS))
    w = np.zeros((D, 2 * TT), dtype=np.float32)
    w[:, 0:HS] = Wq * scale
    w[:, HS:2 * HS] = Wk
    w[:, 2 * HS:3 * HS] = Wv
    # swizzle: w_sb[p, c*256+j] = w[c*128+p, j]
    wsw = np.ascontiguousarray(
        w.reshape(NDT, TT, 2 * TT).transpose(1, 0, 2).reshape(TT, NDT * 2 * TT)
    )
    bcol = np.zeros((TT, 2), dtype=np.float32)
    bcol[0:HS, 0] = bq * scale
    bcol[HS:2 * HS, 0] = bk
    bcol[0:HS, 1] = bv
    bcol[HS, 1] = 1.0
    # xts[s*128+p, c*512+q] = x_b[s*512+q, c*128+p]
    xts = np.ascontiguousarray(
        x_b.reshape(NQS, QS, NDT, TT).transpose(0, 3, 2, 1)
        .reshape(NQS * TT, NDT * QS)
    ).astype(bf)
    mask = np.triu(np.ones((TT, TT), dtype=bf))
    return {
        "xts": xts,
        "wqkv": wsw.astype(bf),
        "bcol": bcol,
        "ident": np.eye(TT, dtype=np.float32),
        "identb": np.eye(TT, dtype=bf),
        "mask": mask,
    }


_NC_CACHE = None


def _get_nc():
    global _NC_CACHE
    if _NC_CACHE is None:
        _NC_CACHE = build_graph()
    return _NC_CACHE


def kernel(x, Wq, bq, Wk, bk, Wv, bv):
    x = np.asarray(x, dtype=np.float32)
    args = [np.asarray(a, dtype=np.float32) for a in (Wq, bq, Wk, bk, Wv, bv)]
    nc = _get_nc()
    in_maps = [make_inputs(x[b], *args) for b in range(N_CORES)]
    trace = os.environ.get("BASS_ATTN_TRACE", "0") == "1"
    res = run_bass_kernel_spmd(
        nc, in_maps, core_ids=list(range(N_CORES)), trace=trace
    )
    if trace:
        print(
            f"HW exec time: {res.exec_time_ns} ns "
            f"(mean {res.mean_exec_time_ns}, max core {res.max_exec_time_core_id})"
        )
    out = np.stack([res.results[b]["out"] for b in range(N_CORES)], axis=0)
    return out
